# revision 2
# baseline (speedup 1.0000x reference)
"""Modulated conv2d (StyleGAN2-style) on 8 TRN2 NeuronCores.

Math: out[b] = conv2d(x[b], w_b), w_b = SCALE*weight*s[b,cin]*pmask[b,cin]*demod[b,cout]
Identity used: per-sample weight modulation folds into per-channel INPUT scaling
(mscale = SCALE*s*pmask) and per-channel OUTPUT scaling (demod), leaving a single
SHARED-weight 3x3 conv -> 9 shifted matmuls accumulated in PSUM.
co = normalized weight gram matrix; each core computes a 64-row slice.

Sharding: data-parallel over batch, 2 samples/core; weight replicated.
"""

import math
import sys

import numpy as np

sys.path.insert(0, "/opt/trn_rl_repo")

import ml_dtypes

B, CIN, COUT, K, SDIM, H, W = 16, 512, 512, 3, 512, 64, 64
SCALE = 1.0 / math.sqrt(CIN * K * K)
LIN_SCALE = 1.0 / math.sqrt(SDIM)
EPS = 1e-8

NCORES = 8
BPC = B // NCORES          # samples per core
P = 128
NCH = CIN // P             # 4 cin chunks
NCO = COUT // P            # 4 cout chunks
KHW = K * K                # 9
HW = H * W                 # 4096
PW = W + 2                 # 66 padded
NSP = 8                    # spatial tiles of 512 (8 image rows each)
ROWS = H // NSP            # 8 rows per spatial tile
GRP = 4                    # psum tiles per group
NG = NSP // GRP            # 2 groups
GROWS = COUT // NCORES     # 64 gram rows per core

TRACE = False              # test.py sets True to profile
_CACHE = {}


def _build_nc():
    import concourse.mybir as mybir
    import concourse.tile as tile
    from concourse import bacc

    nc = bacc.Bacc("TRN2", target_bir_lowering=False, debug=False,
                   num_devices=NCORES)

    x_ext = nc.declare_dram_parameter("x", [BPC, CIN, H, W], mybir.dt.float32,
                                      isOutput=False)
    # weight split per cin chunk: wt_c[c][p, k*COUT + j]
    wt_exts = [
        nc.declare_dram_parameter(f"wt{c}", [P, KHW * COUT],
                                  mybir.dt.bfloat16, isOutput=False)
        for c in range(NCH)
    ]
    wtr_ext = nc.declare_dram_parameter("wtr", [P, NCH * KHW * GROWS],
                                        mybir.dt.bfloat16, isOutput=False)
    ms_ext = nc.declare_dram_parameter("ms", [P, BPC * NCH], mybir.dt.float32,
                                       isOutput=False)
    dm_ext = nc.declare_dram_parameter("dm", [P, BPC * NCO], mybir.dt.float32,
                                       isOutput=False)
    cs_ext = nc.declare_dram_parameter("cs", [GROWS, COUT], mybir.dt.float32,
                                       isOutput=False)
    out_ext = nc.declare_dram_parameter("out", [BPC, COUT, HW],
                                        mybir.dt.float32, isOutput=True)
    gco_ext = nc.declare_dram_parameter("gco", [GROWS, COUT],
                                        mybir.dt.float32, isOutput=True)

    NKC = KHW * NCH  # 36 contraction chunks

    with tile.TileContext(nc) as tc:
        with (
            tc.tile_pool(name="const", bufs=1) as const,
            tc.tile_pool(name="xraw", bufs=3) as xraw_pool,
            tc.tile_pool(name="outp", bufs=8) as out_pool,
            tc.tile_pool(name="psum", bufs=8, space="PSUM") as psum_pool,
        ):
            # persistent padded-input tiles (2 sample sets x 4 chunks);
            # borders zeroed once, interiors rewritten per sample
            xpads = [const.tile([P, PW, PW], mybir.dt.bfloat16,
                                tag=f"xpad{j}", name=f"xpad{j}")
                     for j in range(2 * NCH)]
            for xpt in xpads:
                nc.gpsimd.memset(xpt[:, 0, :], 0.0)
                nc.gpsimd.memset(xpt[:, PW - 1, :], 0.0)
                nc.gpsimd.memset(xpt[:, 1:PW - 1, 0:1], 0.0)
                nc.gpsimd.memset(xpt[:, 1:PW - 1, PW - 1:PW], 0.0)

            ms_sb = const.tile([P, BPC * NCH], mybir.dt.float32)
            nc.sync.dma_start(ms_sb[:], ms_ext[:])

            # sample-0 input first: it gates the first conv matmuls
            def load_sample(b):
                xp = []
                for c in range(NCH):
                    xr = xraw_pool.tile([P, H, W], mybir.dt.float32, tag="xr",
                                        name=f"xr_{b}_{c}")
                    nc.sync.dma_start(xr[:], x_ext[b, c * P:(c + 1) * P])
                    xpt = xpads[(b % 2) * NCH + c]
                    nc.vector.tensor_scalar_mul(
                        xpt[:, 1:H + 1, 1:W + 1], xr[:],
                        ms_sb[:, b * NCH + c: b * NCH + c + 1])
                    xp.append(xpt)
                return xp

            xp0 = load_sample(0)

            wt_sbs = []
            for c in range(NCH):
                wt_c = const.tile([P, KHW * COUT], mybir.dt.bfloat16,
                                  tag=f"wt{c}", name=f"wt{c}")
                nc.sync.dma_start(wt_c[:], wt_exts[c][:])
                wt_sbs.append(wt_c)
            wtr_sb = const.tile([P, NCH * KHW * GROWS], mybir.dt.bfloat16)
            nc.sync.dma_start(wtr_sb[:], wtr_ext[:])
            dm_sb = const.tile([P, BPC * NCO], mybir.dt.float32)
            cs_sb = const.tile([GROWS, COUT], mybir.dt.float32)
            nc.sync.dma_start(dm_sb[:], dm_ext[:])
            nc.sync.dma_start(cs_sb[:], cs_ext[:])

            # weight self-correlation slice: G = Wf[rows]^T-chunks @ Wf
            g_ps = psum_pool.tile([GROWS, COUT], mybir.dt.float32, tag="ps",
                                  name="g_ps")
            for c in range(NCH):
                for k in range(KHW):
                    i = c * KHW + k
                    nc.tensor.matmul(
                        g_ps[:],
                        wtr_sb[:, i * GROWS:(i + 1) * GROWS],
                        wt_sbs[c][:, k * COUT:(k + 1) * COUT],
                        start=(i == 0), stop=(i == NKC - 1),
                    )
            g_sb = out_pool.tile([GROWS, COUT], mybir.dt.float32, tag="out",
                                 name="g_sb")
            nc.vector.tensor_mul(g_sb[:], g_ps[:], cs_sb[:])
            nc.sync.dma_start(gco_ext[:], g_sb[:])

            # conv, one sample at a time
            for b in range(BPC):
                xp = xp0 if b == 0 else load_sample(b)
                for co_c in range(NCO):
                    dmcol = dm_sb[:, b * NCO + co_c: b * NCO + co_c + 1]
                    for g in range(NG):
                        pts = [psum_pool.tile([P, ROWS * W], mybir.dt.float32,
                                              tag="ps", name=f"ps_{b}_{co_c}_{g}_{s}")
                               for s in range(GRP)]
                        ki = 0
                        for c in range(NCH):
                            for k in range(KHW):
                                kh, kw = divmod(k, K)
                                lhsT = wt_sbs[c][:, k * COUT + co_c * P:
                                                 k * COUT + co_c * P + P]
                                for s in range(GRP):
                                    r0 = (g * GRP + s) * ROWS
                                    rhs = xp[c][:, r0 + kh: r0 + kh + ROWS,
                                                kw: kw + W]
                                    nc.tensor.matmul(
                                        pts[s][:], lhsT, rhs,
                                        start=(ki == 0), stop=(ki == NKC - 1))
                                ki += 1
                        for s in range(GRP):
                            ot = out_pool.tile([P, ROWS * W], mybir.dt.float32,
                                               tag="out", name=f"ot_{b}_{co_c}_{g}_{s}")
                            nc.vector.tensor_scalar_mul(ot[:], pts[s][:], dmcol)
                            r0 = (g * GRP + s) * ROWS
                            nc.sync.dma_start(
                                out_ext[b, co_c * P:(co_c + 1) * P,
                                        r0 * W:(r0 + ROWS) * W],
                                ot[:])
    nc.finalize()
    return nc


def _prep_host(input, prob, style, weight, mod_weight, mod_bias):
    x = np.asarray(input, np.float32)
    p = np.asarray(prob, np.float64)
    st = np.asarray(style, np.float64)
    w = np.asarray(weight, np.float64)[0]          # (COUT, CIN, K, K)
    mw = np.asarray(mod_weight, np.float64)
    mb = np.asarray(mod_bias, np.float64)

    s = st @ (mw * LIN_SCALE).T + mb               # (B, CIN)
    thresh = np.floor(np.round(p * 8.0) / 8.0 * CIN * 0.5)
    pmask = (np.arange(CIN)[None, :] >= thresh[:, None]).astype(np.float64)
    mscale = (SCALE * s * pmask)                   # (B, CIN)

    ws2 = np.sum(w * w, axis=(2, 3))               # (COUT, CIN)
    demod = 1.0 / np.sqrt((mscale * mscale) @ ws2.T + EPS)   # (B, COUT)

    norm = np.sqrt(np.sum(ws2, axis=1))            # (COUT,)
    invn = 1.0 / np.maximum(norm, 1e-12)

    # weight -> [kh,kw,cin,cout] -> chunked [k, c, p, cout], bf16
    wt = w.transpose(2, 3, 1, 0).reshape(KHW, NCH, P, COUT).astype(np.float32)
    wt_devs = [
        np.ascontiguousarray(
            wt[:, c].transpose(1, 0, 2).reshape(P, KHW * COUT)
        ).astype(ml_dtypes.bfloat16)
        for c in range(NCH)
    ]

    ms_all = mscale.astype(np.float32).reshape(B, NCH, P)
    dm_all = demod.astype(np.float32).reshape(B, NCO, P)

    in_maps = []
    for i in range(NCORES):
        b0 = i * BPC
        # wtr col index (c*KHW + k)*GROWS + j
        wtr_dev = np.ascontiguousarray(
            wt[:, :, :, i * GROWS:(i + 1) * GROWS]
            .transpose(2, 1, 0, 3).reshape(P, NCH * KHW * GROWS)
        ).astype(ml_dtypes.bfloat16)
        cs_dev = np.ascontiguousarray(
            (invn[i * GROWS:(i + 1) * GROWS, None] * invn[None, :])
            .astype(np.float32))
        m = {
            "x": np.ascontiguousarray(x[b0:b0 + BPC]),
            "wtr": wtr_dev,
            "ms": np.ascontiguousarray(
                ms_all[b0:b0 + BPC].transpose(2, 0, 1).reshape(P, BPC * NCH)),
            "dm": np.ascontiguousarray(
                dm_all[b0:b0 + BPC].transpose(2, 0, 1).reshape(P, BPC * NCO)),
            "cs": cs_dev,
        }
        for c in range(NCH):
            m[f"wt{c}"] = wt_devs[c]
        in_maps.append(m)
    return in_maps


def kernel(input, prob, style, weight, mod_weight, mod_bias):
    from concourse.bass_utils import run_bass_kernel_spmd

    if "nc" not in _CACHE:
        _CACHE["nc"] = _build_nc()
    nc = _CACHE["nc"]

    in_maps = _prep_host(input, prob, style, weight, mod_weight, mod_bias)
    res = run_bass_kernel_spmd(nc, in_maps, list(range(NCORES)), trace=TRACE)
    _CACHE["last_result"] = res

    out = np.concatenate(
        [res.results[i]["out"].reshape(BPC, COUT, H, W) for i in range(NCORES)],
        axis=0)
    co = np.concatenate(
        [res.results[i]["gco"] for i in range(NCORES)], axis=0)[None]
    return out.astype(np.float32), co.astype(np.float32)


# revision 3
# speedup vs baseline: 1.0548x; 1.0548x over previous
"""Modulated conv2d (StyleGAN2-style) on 8 TRN2 NeuronCores.

Math: out[b] = conv2d(x[b], w_b), w_b = SCALE*weight*s[b,cin]*pmask[b,cin]*demod[b,cout]
Identity used: per-sample weight modulation folds into per-channel INPUT scaling
(mscale = SCALE*s*pmask) and per-channel OUTPUT scaling (demod), leaving a single
SHARED-weight 3x3 conv -> 9 shifted matmuls accumulated in PSUM.
co = normalized weight gram matrix; each core computes a 64-row slice.

Sharding: data-parallel over batch, 2 samples/core; weight replicated.
"""

import math
import sys

import numpy as np

sys.path.insert(0, "/opt/trn_rl_repo")

import ml_dtypes

B, CIN, COUT, K, SDIM, H, W = 16, 512, 512, 3, 512, 64, 64
SCALE = 1.0 / math.sqrt(CIN * K * K)
LIN_SCALE = 1.0 / math.sqrt(SDIM)
EPS = 1e-8

NCORES = 8
BPC = B // NCORES          # samples per core
P = 128
NCH = CIN // P             # 4 cin chunks
NCO = COUT // P            # 4 cout chunks
KHW = K * K                # 9
HW = H * W                 # 4096
PW = W + 2                 # 66 padded
NSP = 8                    # spatial tiles of 512 (8 image rows each)
ROWS = H // NSP            # 8 rows per spatial tile
GRP = 4                    # psum tiles per group
NG = NSP // GRP            # 2 groups
GROWS = COUT // NCORES     # 64 gram rows per core

TRACE = False              # test.py sets True to profile
_CACHE = {}


def _build_nc():
    import concourse.mybir as mybir
    import concourse.tile as tile
    from concourse import bacc

    nc = bacc.Bacc("TRN2", target_bir_lowering=False, debug=False,
                   num_devices=NCORES)

    x_ext = nc.declare_dram_parameter("x", [BPC, CIN, H, W], mybir.dt.float32,
                                      isOutput=False)
    # weight split per cin chunk: wt_c[c][p, k*COUT + j]
    wt_exts = [
        nc.declare_dram_parameter(f"wt{c}", [P, KHW * COUT],
                                  mybir.dt.bfloat16, isOutput=False)
        for c in range(NCH)
    ]
    wtr_ext = nc.declare_dram_parameter("wtr", [P, NCH * KHW * GROWS],
                                        mybir.dt.bfloat16, isOutput=False)
    ms_ext = nc.declare_dram_parameter("ms", [P, BPC * NCH], mybir.dt.float32,
                                       isOutput=False)
    dm_ext = nc.declare_dram_parameter("dm", [P, BPC * NCO], mybir.dt.float32,
                                       isOutput=False)
    cs_ext = nc.declare_dram_parameter("cs", [GROWS, COUT], mybir.dt.float32,
                                       isOutput=False)
    out_ext = nc.declare_dram_parameter("out", [BPC, COUT, HW],
                                        mybir.dt.float32, isOutput=True)
    gco_ext = nc.declare_dram_parameter("gco", [GROWS, COUT],
                                        mybir.dt.float32, isOutput=True)

    NKC = KHW * NCH  # 36 contraction chunks

    with tile.TileContext(nc) as tc:
        with (
            tc.tile_pool(name="const", bufs=1) as const,
            tc.tile_pool(name="xraw", bufs=3) as xraw_pool,
            tc.tile_pool(name="outp", bufs=8) as out_pool,
            tc.tile_pool(name="psum", bufs=8, space="PSUM") as psum_pool,
        ):
            # persistent padded-input tiles (2 sample sets x 4 chunks);
            # borders zeroed once, interiors rewritten per sample
            xpads = [const.tile([P, PW, PW], mybir.dt.bfloat16,
                                tag=f"xpad{j}", name=f"xpad{j}")
                     for j in range(2 * NCH)]
            for xpt in xpads:
                nc.gpsimd.memset(xpt[:, 0, :], 0.0)
                nc.gpsimd.memset(xpt[:, PW - 1, :], 0.0)
                nc.gpsimd.memset(xpt[:, 1:PW - 1, 0:1], 0.0)
                nc.gpsimd.memset(xpt[:, 1:PW - 1, PW - 1:PW], 0.0)

            ms_sb = const.tile([P, BPC * NCH], mybir.dt.float32)
            nc.sync.dma_start(ms_sb[:], ms_ext[:])

            # weights first: they gate the gram matmuls (first PE phase)
            wtr_sb = const.tile([P, NCH * KHW * GROWS], mybir.dt.bfloat16)
            nc.sync.dma_start(wtr_sb[:], wtr_ext[:])
            wt_sbs = []
            for c in range(NCH):
                wt_c = const.tile([P, KHW * COUT], mybir.dt.bfloat16,
                                  tag=f"wt{c}", name=f"wt{c}")
                nc.sync.dma_start(wt_c[:], wt_exts[c][:])
                wt_sbs.append(wt_c)

            def load_sample(b):
                xp = []
                for c in range(NCH):
                    xr = xraw_pool.tile([P, H, W], mybir.dt.float32, tag="xr",
                                        name=f"xr_{b}_{c}")
                    nc.sync.dma_start(xr[:], x_ext[b, c * P:(c + 1) * P])
                    xpt = xpads[(b % 2) * NCH + c]
                    nc.vector.tensor_scalar_mul(
                        xpt[:, 1:H + 1, 1:W + 1], xr[:],
                        ms_sb[:, b * NCH + c: b * NCH + c + 1])
                    xp.append(xpt)
                return xp

            xp0 = load_sample(0)

            dm_sb = const.tile([P, BPC * NCO], mybir.dt.float32)
            cs_sb = const.tile([GROWS, COUT], mybir.dt.float32)
            nc.sync.dma_start(dm_sb[:], dm_ext[:])
            nc.sync.dma_start(cs_sb[:], cs_ext[:])

            # weight self-correlation slice: G = Wf[rows]^T-chunks @ Wf
            g_ps = psum_pool.tile([GROWS, COUT], mybir.dt.float32, tag="ps",
                                  name="g_ps")
            for c in range(NCH):
                for k in range(KHW):
                    i = c * KHW + k
                    nc.tensor.matmul(
                        g_ps[:],
                        wtr_sb[:, i * GROWS:(i + 1) * GROWS],
                        wt_sbs[c][:, k * COUT:(k + 1) * COUT],
                        start=(i == 0), stop=(i == NKC - 1),
                    )
            g_sb = out_pool.tile([GROWS, COUT], mybir.dt.float32, tag="out",
                                 name="g_sb")
            nc.vector.tensor_mul(g_sb[:], g_ps[:], cs_sb[:])
            nc.sync.dma_start(gco_ext[:], g_sb[:])

            # conv, one sample at a time
            for b in range(BPC):
                xp = xp0 if b == 0 else load_sample(b)
                for co_c in range(NCO):
                    dmcol = dm_sb[:, b * NCO + co_c: b * NCO + co_c + 1]
                    for g in range(NG):
                        pts = [psum_pool.tile([P, ROWS * W], mybir.dt.float32,
                                              tag="ps", name=f"ps_{b}_{co_c}_{g}_{s}")
                               for s in range(GRP)]
                        ki = 0
                        for c in range(NCH):
                            for k in range(KHW):
                                kh, kw = divmod(k, K)
                                lhsT = wt_sbs[c][:, k * COUT + co_c * P:
                                                 k * COUT + co_c * P + P]
                                for s in range(GRP):
                                    r0 = (g * GRP + s) * ROWS
                                    rhs = xp[c][:, r0 + kh: r0 + kh + ROWS,
                                                kw: kw + W]
                                    nc.tensor.matmul(
                                        pts[s][:], lhsT, rhs,
                                        start=(ki == 0), stop=(ki == NKC - 1))
                                ki += 1
                        for s in range(GRP):
                            ot = out_pool.tile([P, ROWS * W], mybir.dt.float32,
                                               tag="out", name=f"ot_{b}_{co_c}_{g}_{s}")
                            nc.vector.tensor_scalar_mul(ot[:], pts[s][:], dmcol)
                            r0 = (g * GRP + s) * ROWS
                            nc.sync.dma_start(
                                out_ext[b, co_c * P:(co_c + 1) * P,
                                        r0 * W:(r0 + ROWS) * W],
                                ot[:])
    nc.finalize()
    return nc


def _prep_host(input, prob, style, weight, mod_weight, mod_bias):
    x = np.asarray(input, np.float32)
    p = np.asarray(prob, np.float64)
    st = np.asarray(style, np.float64)
    w = np.asarray(weight, np.float64)[0]          # (COUT, CIN, K, K)
    mw = np.asarray(mod_weight, np.float64)
    mb = np.asarray(mod_bias, np.float64)

    s = st @ (mw * LIN_SCALE).T + mb               # (B, CIN)
    thresh = np.floor(np.round(p * 8.0) / 8.0 * CIN * 0.5)
    pmask = (np.arange(CIN)[None, :] >= thresh[:, None]).astype(np.float64)
    mscale = (SCALE * s * pmask)                   # (B, CIN)

    ws2 = np.sum(w * w, axis=(2, 3))               # (COUT, CIN)
    demod = 1.0 / np.sqrt((mscale * mscale) @ ws2.T + EPS)   # (B, COUT)

    norm = np.sqrt(np.sum(ws2, axis=1))            # (COUT,)
    invn = 1.0 / np.maximum(norm, 1e-12)

    # weight -> [kh,kw,cin,cout] -> chunked [k, c, p, cout], bf16
    wt = w.transpose(2, 3, 1, 0).reshape(KHW, NCH, P, COUT).astype(np.float32)
    wt_devs = [
        np.ascontiguousarray(
            wt[:, c].transpose(1, 0, 2).reshape(P, KHW * COUT)
        ).astype(ml_dtypes.bfloat16)
        for c in range(NCH)
    ]

    ms_all = mscale.astype(np.float32).reshape(B, NCH, P)
    dm_all = demod.astype(np.float32).reshape(B, NCO, P)

    in_maps = []
    for i in range(NCORES):
        b0 = i * BPC
        # wtr col index (c*KHW + k)*GROWS + j
        wtr_dev = np.ascontiguousarray(
            wt[:, :, :, i * GROWS:(i + 1) * GROWS]
            .transpose(2, 1, 0, 3).reshape(P, NCH * KHW * GROWS)
        ).astype(ml_dtypes.bfloat16)
        cs_dev = np.ascontiguousarray(
            (invn[i * GROWS:(i + 1) * GROWS, None] * invn[None, :])
            .astype(np.float32))
        m = {
            "x": np.ascontiguousarray(x[b0:b0 + BPC]),
            "wtr": wtr_dev,
            "ms": np.ascontiguousarray(
                ms_all[b0:b0 + BPC].transpose(2, 0, 1).reshape(P, BPC * NCH)),
            "dm": np.ascontiguousarray(
                dm_all[b0:b0 + BPC].transpose(2, 0, 1).reshape(P, BPC * NCO)),
            "cs": cs_dev,
        }
        for c in range(NCH):
            m[f"wt{c}"] = wt_devs[c]
        in_maps.append(m)
    return in_maps


def kernel(input, prob, style, weight, mod_weight, mod_bias):
    from concourse.bass_utils import run_bass_kernel_spmd

    if "nc" not in _CACHE:
        _CACHE["nc"] = _build_nc()
    nc = _CACHE["nc"]

    in_maps = _prep_host(input, prob, style, weight, mod_weight, mod_bias)
    res = run_bass_kernel_spmd(nc, in_maps, list(range(NCORES)), trace=TRACE)
    _CACHE["last_result"] = res

    out = np.concatenate(
        [res.results[i]["out"].reshape(BPC, COUT, H, W) for i in range(NCORES)],
        axis=0)
    co = np.concatenate(
        [res.results[i]["gco"] for i in range(NCORES)], axis=0)[None]
    return out.astype(np.float32), co.astype(np.float32)
